# revision 1
# baseline (speedup 1.0000x reference)
"""Viterbi CRF decode on Trainium2 (Bass), 8-core data-parallel.

Problem: B=128, S=512, T=32 (30 labels + START=30, END=31).
  forward max-plus scan over S steps, backpointers, masked lengths,
  backward pointer-following pass. Output [B, S] int32 tag path.

Sharding: pure data parallel, 16 examples per core.

Per-core layout (SBUF partitions p = 32*q + j, quadrant q in [0,4) holds
examples b = 4q+br, br in [0,4); j in [0,32) is the tag index):
  - state P4[p, (br,i)] = part[b, i] (part vector replicated across the 32
    j-partitions of each quadrant)
  - per step: scores = feats+trans (bcast APs), vals = scores + P4,
    segmented max-reduce over i -> part history PH[:, 4t+br],
    eq/iota-desc/max-reduce -> backpointer history (first-argmax encoded
    as 31-i), then a 32x32 block transpose + 4 broadcast stream_shuffles
    rebuild P4 for the next step.
  - pointer phase: arithmetic select of part at last valid position
    (monotone mask -> at-last indicator), argmax into END tag.
  - backward: per step, block-transpose of the bp row + one fused
    scalar_tensor_tensor (one-hot select, sum-accumulate) = the gather.

All compute on the vector engine (exact fp32, same association order as
the jax reference: (feats + trans) + part), DMA on sync engine.
"""

import numpy as np
from contextlib import ExitStack

import concourse.bass as bass
import concourse.mybir as mybir
from concourse.bass_utils import run_bass_kernel_spmd

F32 = mybir.dt.float32
I32 = mybir.dt.int32
AX = mybir.AxisListType
OP = mybir.AluOpType

T = 32
START = 30
END = 31
NCORES = 8


def build_nc(S, debug=False, reps=1, skip_bp=False, skip_move=False, sim_compat=False):
    # Single compute engine (DVE) in program order: same-engine RAW/WAW is
    # serialized by the hardware (per-op pipe drain); the conservative race
    # detector does not model engine ordering, so it is disabled.
    nc = bass.Bass(detect_race_conditions=False)
    ft_d = nc.declare_dram_parameter("ft", [128, 4 * S], F32, isOutput=False)
    mkf_d = nc.declare_dram_parameter("mkf", [128, 4 * S + 4], F32, isOutput=False)
    tt_d = nc.declare_dram_parameter("tt", [128, 32], F32, isOutput=False)
    cst_d = nc.declare_dram_parameter("cst", [128, 64], F32, isOutput=False)
    dec_d = nc.declare_dram_parameter("dec", [128, S], I32, isOutput=True)
    if debug:
        ph_d = nc.declare_dram_parameter("d_ph", [128, 4 * S + 32], F32, isOutput=True)
        bpw_d = nc.declare_dram_parameter("d_bpw", [128, 4 * S + 32], F32, isOutput=True)
        bpf_d = nc.declare_dram_parameter("d_bpf", [128, 4 * S + 32], F32, isOutput=True)
        decf_d = nc.declare_dram_parameter("d_decf", [128, S], F32, isOutput=True)
        p4_d = nc.declare_dram_parameter("d_p4", [128, 128], F32, isOutput=True)
        p32_d = nc.declare_dram_parameter("d_p32", [128, 32], F32, isOutput=True)
        lpp_d = nc.declare_dram_parameter("d_lpp", [128, 32], F32, isOutput=True)

    K = S - 1  # bp rows k in [0, K)

    with ExitStack() as ctx:
        def sb(name, shape, dt=F32):
            return ctx.enter_context(nc.sbuf_tensor(name, shape, dt))

        FT = sb("FT", [128, 4 * S])
        MKF = sb("MKF", [128, 4 * S + 4])
        TT = sb("TT", [128, 32])
        PH = sb("PH", [128, 4 * S + 32])
        BPW = sb("BPW", [128, 4 * S + 32])
        XS = sb("XS", [128, 4 * S + 32])
        XS2 = sb("XS2", [128, 4 * S + 32])
        ALF = sb("ALF", [128, 4 * S])
        ALB = sb("ALB", [128, 4 * S])
        SCH = sb("SCH", [128, 64 * 128])
        WB0 = sb("WB0", [128, 128])
        WB1 = sb("WB1", [128, 128])
        WB2 = sb("WB2", [128, 128])
        WB3 = sb("WB3", [128, 128])
        P4 = sb("P4", [128, 128])
        S4 = sb("S4", [128, 128])
        V = sb("V", [128, 128])
        T32 = sb("T32", [128, 32])
        DEC = sb("DEC", [128, S])
        DECI = sb("DECI", [128, S], I32)
        CST = sb("CST", [128, 64])
        TEND = sb("TEND", [128, 32])
        LPP = sb("LPP", [128, 32])
        TLP = sb("TLP", [128, 32])
        CAND = sb("CAND", [128, 32])
        MX = sb("MX", [128, 1])
        EQC = sb("EQC", [128, 32])
        PW = sb("PW", [128, 1])
        P32 = sb("P32", [128, 32])
        PR = sb("PR", [128, 32])
        SC = sb("SC", [128, 32])

        with (
            nc.semaphore() as dma_sem,
            nc.semaphore() as done_sem,
            nc.Block() as block,
        ):
            @block.sync
            def _(sync):
                sync.dma_start(out=FT[:], in_=ft_d[:]).then_inc(dma_sem, 16)
                sync.dma_start(out=MKF[:], in_=mkf_d[:]).then_inc(dma_sem, 16)
                sync.dma_start(out=TT[:], in_=tt_d[:]).then_inc(dma_sem, 16)
                sync.dma_start(out=CST[:], in_=cst_d[:]).then_inc(dma_sem, 16)
                sync.wait_ge(done_sem, 1)
                sync.dma_start(out=dec_d[:], in_=DECI[:]).then_inc(dma_sem, 16)
                if debug:
                    sync.dma_start(out=ph_d[:], in_=PH[:]).then_inc(dma_sem, 16)
                    sync.dma_start(out=bpw_d[:], in_=BPW[:]).then_inc(dma_sem, 16)
                    sync.dma_start(out=bpf_d[:], in_=XS[:]).then_inc(dma_sem, 16)
                    sync.dma_start(out=decf_d[:], in_=DEC[:]).then_inc(dma_sem, 16)
                    sync.dma_start(out=p4_d[:], in_=P4[:]).then_inc(dma_sem, 16)
                    sync.dma_start(out=p32_d[:], in_=P32[:]).then_inc(dma_sem, 16)
                    sync.dma_start(out=lpp_d[:], in_=LPP[:]).then_inc(dma_sem, 16)

            def emit_body(v, rep):
                base = rep * 512
                # constants / scratch init
                v.stream_shuffle(out=TEND[:], in_=TT[:], mask=[END] * 32)
                v.memset(PH[:], 0.0)
                v.memset(XS[:, 4 * K:], 0.0)
                v.memset(BPW[:, 4 * K:], 0.0)
                v.memset(P32[:], 0.0)
                v.memset(LPP[:], 0.0)

                # init t=0: part0[b, j] = feats[b,0,j] + trans[START, j]
                v.tensor_scalar_add(out=PH[:, 0:4], in0=FT[:, 0:4],
                                    scalar1=TT[:, START:START + 1])
                # independent fillers: give the PH write time to land before
                # the transpose reads it (HW has no end-write->read interlock)
                v.tensor_sub(out=ALF[:], in0=MKF[:, 0:4 * S], in1=MKF[:, 4:4 * S + 4])
                v.tensor_scalar(out=ALB[:], in0=ALF[:], scalar1=1.0,
                                scalar2=1e30, op0=OP.subtract, op1=OP.mult)
                # bulk scores: SCH[p, 128*u + 32*br + i] = feats[b,t0+u,j] + trans[i,j]
                tt_c = TT[:].unsqueeze(1).unsqueeze(1).broadcast_to([128, 64, 4, 32])
                sch_v = SCH[:].rearrange("p (u b i) -> p u b i", b=4, i=32)

                def sch_chunk(c):
                    ft_c = FT[:, 256 * c:256 * (c + 1)].rearrange(
                        "p (u b) -> p u b", b=4).unsqueeze(3).broadcast_to([128, 64, 4, 32])
                    v.tensor_tensor(out=sch_v, in0=ft_c, in1=tt_c, op=OP.add)

                sch_chunk(0)
                p4_blk = P4[:].rearrange("p (b i) -> p b i", b=4)

                def p4_build(t0):
                    # replicate part_t0[b,:] to every partition of its quadrant:
                    # one 4-block stream-transpose with 0-stride input columns
                    if sim_compat:
                        for br in range(4):
                            v.transpose(out=P4[:, 32 * br:32 * br + 32],
                                        in_=PH[:, 4 * t0 + br:4 * t0 + br + 1].broadcast_to([128, 32]))
                    else:
                        v.transpose(out=p4_blk,
                                    in_=PH[:, 4 * t0:4 * t0 + 4].unsqueeze(2).broadcast_to([128, 4, 32]))

                p4_build(0)

                V2 = [V, S4]   # double-buffered vals: slot t%2 holds V_t
                iotad_b = CST[:, 32:64].unsqueeze(1).broadcast_to([128, 4, 32])
                w_v = WB0[:].rearrange("p (b i) -> p b i", b=4)
                eb_v = WB1[:].rearrange("p (b i) -> p b i", b=4)

                def eq_op(tp):
                    vp3 = V2[tp % 2][:].rearrange("p (b i) -> p b i", b=4)
                    php = PH[:, 4 * tp:4 * tp + 4].unsqueeze(2).broadcast_to([128, 4, 32])
                    v.tensor_tensor(out=eb_v, in0=vp3, in1=php, op=OP.is_equal)

                def mul_op():
                    v.tensor_tensor(out=w_v, in0=eb_v, in1=iotad_b, op=OP.mult)

                def red_op(tp):
                    v.tensor_reduce(out=BPW[:, 4 * (tp - 1):4 * (tp - 1) + 4],
                                    in_=w_v, axis=AX.X, op=OP.max)

                # forward scan: per step
                #   [eq(t-1), Vadd_t, reduce_t, mul(t-1), bpwred(t-1), transp x4]
                # eq/mul serve as the fillers that keep an op of distance
                # between end-of-stream writes and start-of-stream reads.
                for t in range(1, S):
                    u = t % 64
                    if u == 0:
                        sch_chunk(t // 64)
                    if not skip_bp and t >= 2:
                        eq_op(t - 1)
                    else:
                        v.drain()
                    vc = V2[t % 2][:]
                    v.tensor_tensor(out=vc, in0=SCH[:, 128 * u:128 * u + 128],
                                    in1=P4[:], op=OP.add)
                    v.tensor_reduce(out=PH[:, 4 * t:4 * t + 4],
                                    in_=vc.rearrange("p (b i) -> p b i", b=4),
                                    axis=AX.X, op=OP.max)
                    if not skip_bp and t >= 2:
                        mul_op()
                        red_op(t - 1)
                    else:
                        v.drain()
                        v.drain()
                    if t < S - 1 and not skip_move:
                        p4_build(t)
                if not skip_bp:
                    eq_op(S - 1)
                    v.drain()
                    mul_op()
                    v.drain()
                    red_op(S - 1)

                # last_partition by-i-partition: max over t of PH + ALB
                ph_bt = PH[:, 0:4 * S].rearrange("p (t b) -> p b t", b=4)
                alb_bt = ALB[:].rearrange("p (t b) -> p b t", b=4)
                xs_bt = XS[:, 0:4 * S].rearrange("p (t b) -> p b t", b=4)
                v.tensor_tensor(out=xs_bt, in0=ph_bt, in1=alb_bt, op=OP.add)
                v.tensor_reduce(out=LPP[:, 0:4], in_=xs_bt, axis=AX.X, op=OP.max)

                # bp decode + mask (independent of LPP; also serves as filler)
                v.tensor_scalar(out=XS2[:, 0:4 * K], in0=BPW[:, 0:4 * K],
                                scalar1=-1.0, scalar2=31.0, op0=OP.mult, op1=OP.add)
                v.tensor_tensor(out=BPW[:, 0:4 * K], in0=XS2[:, 0:4 * K],
                                in1=MKF[:, 4:4 * K + 4], op=OP.mult)

                # pointer = argmax_i(LP[b,i] + trans[i,END]); one-time tail,
                # explicit drains around every end-write -> start-read pair
                v.transpose(out=TLP[:], in_=LPP[:])
                v.drain()
                v.tensor_tensor(out=CAND[:], in0=TLP[:], in1=TEND[:], op=OP.add)
                v.tensor_reduce(out=MX[:], in_=CAND[:], axis=AX.X, op=OP.max)
                v.drain()
                v.tensor_tensor(out=EQC[:], in0=CAND[:],
                                in1=MX[:].broadcast_to([128, 32]), op=OP.is_equal)
                v.tensor_tensor(out=SC[:], in0=EQC[:], in1=CST[:, 32:64], op=OP.mult)
                v.tensor_reduce(out=PW[:], in_=SC[:], axis=AX.X, op=OP.max)
                v.drain()
                v.tensor_scalar(out=P32[:, 0:1], in0=PW[:], scalar1=-1.0,
                                scalar2=31.0, op0=OP.mult, op1=OP.add)
                v.drain()

                # scatter pointer at k == last_pos: bp' = bp + atlast*(ptr - bp)
                v.transpose(out=T32[:], in_=P32[:])
                v.stream_shuffle(out=PR[:], in_=T32[:], mask=[0] * 32)
                v.drain()
                pr_b = PR[:, 0:4].unsqueeze(1).broadcast_to([128, K, 4])
                bp_v = BPW[:, 0:4 * K].rearrange("p (k b) -> p k b", b=4)
                xs_v = XS[:, 0:4 * K].rearrange("p (k b) -> p k b", b=4)
                xs2_v = XS2[:, 0:4 * K].rearrange("p (k b) -> p k b", b=4)
                alf_v = ALF[:, 0:4 * K].rearrange("p (k b) -> p k b", b=4)
                v.tensor_tensor(out=xs_v, in0=pr_b, in1=bp_v, op=OP.subtract)
                v.tensor_tensor(out=xs2_v, in0=xs_v, in1=alf_v, op=OP.mult)
                v.tensor_tensor(out=xs_v, in0=bp_v, in1=xs2_v, op=OP.add)

                # backward pass: transposes pipelined 2 steps ahead of the stt
                v.tensor_copy(out=DEC[:, S - 1:S], in_=P32[:, 0:1])
                TB = [T32, TLP, SC]  # ring of transpose buffers
                v.transpose(out=TB[(S - 2) % 3][:], in_=XS[:, 4 * (S - 2):4 * (S - 2) + 32])
                v.transpose(out=TB[(S - 3) % 3][:], in_=XS[:, 4 * (S - 3):4 * (S - 3) + 32])
                for k in range(S - 2, -1, -1):
                    if k >= 2:
                        v.transpose(out=TB[(k - 2) % 3][:],
                                    in_=XS[:, 4 * (k - 2):4 * (k - 2) + 32])
                    else:
                        v.drain()
                    v.scalar_tensor_tensor(out=EQC[:], in0=CST[:, 0:32],
                                           scalar=DEC[:, k + 1:k + 2],
                                           in1=TB[k % 3][:],
                                           op0=OP.is_equal, op1=OP.mult,
                                           accum_out=DEC[:, k:k + 1])

                v.drain()
                v.tensor_copy(out=DECI[:], in_=DEC[:])
            @block.vector
            def _(v):
                v.wait_ge(dma_sem, 64)
                for _rep in range(reps):
                    emit_body(v, _rep)
                v.drain().then_inc(done_sem, 1)

    return nc


def pack_inputs(feats, transitions, mask, S):
    """Host-side layout packing (pure data movement, no arithmetic beyond
    dtype conversion of the 0/1 mask)."""
    trans = np.ascontiguousarray(np.asarray(transitions, np.float32))
    ttrep = np.ascontiguousarray(np.tile(trans.T, (4, 1)))  # [128, 32]
    iota = np.arange(32, dtype=np.float32)
    cst = np.ascontiguousarray(
        np.tile(np.concatenate([iota, 31.0 - iota])[None, :], (128, 1)))
    in_maps = []
    bc = 16
    for c in range(NCORES):
        f = np.asarray(feats[bc * c:bc * c + bc], np.float32)  # [16, S, 32]
        ft = np.ascontiguousarray(
            f.reshape(4, 4, S, T).transpose(0, 3, 2, 1).reshape(128, 4 * S))
        m = np.asarray(mask[bc * c:bc * c + bc]).astype(np.float32)  # [16, S]
        mk = np.broadcast_to(
            m.reshape(4, 1, 4, S).transpose(0, 1, 3, 2), (4, 32, S, 4))
        mk = mk.reshape(128, 4 * S)
        mkp = np.zeros((128, 4 * S + 4), np.float32)
        mkp[:, :4 * S] = mk
        in_maps.append({"ft": ft, "mkf": mkp, "tt": ttrep, "cst": cst})
    return in_maps


def unpack_outputs(results, S):
    out = np.empty((128, S), np.int32)
    bc = 16
    for c in range(NCORES):
        d = np.asarray(results[c]["dec"]).reshape(4, 32, S)
        out[bc * c:bc * c + bc] = d[:, 0:4, :].reshape(16, S)
    return out


_NC_CACHE = {}


def kernel(feats, transitions, mask):
    B, S, Tin = feats.shape
    assert (B, Tin) == (128, 32)
    if S not in _NC_CACHE:
        _NC_CACHE[S] = build_nc(S)
    nc = _NC_CACHE[S]
    in_maps = pack_inputs(feats, transitions, mask, S)
    res = run_bass_kernel_spmd(nc, in_maps, list(range(NCORES)))
    return unpack_outputs(res.results, S)



# revision 2
# speedup vs baseline: 2.3406x; 2.3406x over previous
"""Viterbi CRF decode on Trainium2 (Bass), 8-core data-parallel — v2.

Per core, 16 examples b = 4q+br (q = quadrant, br in [0,4)); tags T=32.

Forward scan (2 DVE ops + 1 drain per step):
  State kept TAG-ON-PARTITION: PH[(q,i), (t,br)] = part_t[b, i].
  V[(q,i), (br,j)] = SCHW + PH[t-1]-bcast; then tensor_reduce with
  apply_transpose (input transposed in 32x32 blocks per quadrant):
    PH[(q,x), (t,br)] = max_y V[(q,y), (br,x)]  -- new part, tag-on-partition.
  Score windows: every 32 steps one wide broadcast-transpose of FT gives
  feats flat (FTJ[(q,*), (t,br,j)]), + TTI -> SCHW = feats+trans.

Backpointers: batched per 32-step window (one window of lag):
  PWIN = broadcast-transpose of PH cols (part flat); cand = (FT+TTJ)+PWIN
  (bit-identical association to the scan's values); eq vs PH;
  first-argmax via (eq*-4096)+(iota+4096), min-reduce -> BPU8 (u8).

Pointer phase: masked last-position select (exact +0.0 at last position),
argmax into END with the same first-index-min trick.

Backward: pointer-following maps as u8 rows G[(q,x), (k,br)] (masked bp
rows, const-pointer row at k=last_pos, identity pad at k=S-1).
3 levels of pair-composition: left map made y-flat by a u8
broadcast-transpose, one-hot eq*mul+sum-reduce composes. Chase tables
(per-partition example-selected) built from quadrant-flat transposes by
a one-hot br-select (mul+reduce). 64-step chase (stt one-hot gather with
accum), then 3 levels of batched back-substitution. All on the DVE; the
only DMAs in the program are the 6 input loads and 1 output store.
"""

import numpy as np
from contextlib import ExitStack

import concourse.bass as bass
import concourse.mybir as mybir
from concourse.bass_utils import run_bass_kernel_spmd

F32 = mybir.dt.float32
I32 = mybir.dt.int32
U8 = mybir.dt.uint8
AX = mybir.AxisListType
OP = mybir.AluOpType

T = 32
START = 30
END = 31
NCORES = 8
BIG = 4096.0


def build_nc2(S, sim_compat=False, debug=False, reps=1, phase='all',
              no_drain=False):
    assert S % 8 == 0
    nc = bass.Bass(detect_race_conditions=False)
    ft_d = nc.declare_dram_parameter("ft", [128, 4 * S], F32, isOutput=False)
    mkf_d = nc.declare_dram_parameter("mkf", [128, 4 * S + 4], F32, isOutput=False)
    tti_d = nc.declare_dram_parameter("tti", [128, 32], F32, isOutput=False)
    ttj_d = nc.declare_dram_parameter("ttj", [128, 32], F32, isOutput=False)
    cst_d = nc.declare_dram_parameter("cst", [128, 104], F32, isOutput=False)
    cu8_d = nc.declare_dram_parameter("cu8", [128, 64], U8, isOutput=False)
    dec_d = nc.declare_dram_parameter("dec", [128, S], I32, isOutput=True)
    if debug:
        dph_d = nc.declare_dram_parameter("d_ph", [128, 4 * S], F32, isOutput=True)
        dbp_d = nc.declare_dram_parameter("d_bp", [128, 4 * S], U8, isOutput=True)
        dg_d = nc.declare_dram_parameter("d_g", [128, 4 * S], U8, isOutput=True)
        dh1_d = nc.declare_dram_parameter("d_h1", [128, 2 * S], U8, isOutput=True)
        dh3_d = nc.declare_dram_parameter("d_h3", [128, S // 2], U8, isOutput=True)
        ddec_d = nc.declare_dram_parameter("d_dec", [128, S + 8], F32, isOutput=True)
        dgt3_d = nc.declare_dram_parameter("d_gt3", [128, (S // 8) * 32], F32,
                                           isOutput=True)

    NW = (S - 1 + 31) // 32          # 32-step scan windows
    NM = S // 8                      # chase chunks

    with ExitStack() as ctx:
        def sb(name, shape, dt=F32):
            return ctx.enter_context(nc.sbuf_tensor(name, shape, dt))

        FT = sb("FT", [128, 4 * S])
        MKF = sb("MKF", [128, 4 * S + 4])
        TTI = sb("TTI", [128, 32])
        TTJ = sb("TTJ", [128, 32])
        CST = sb("CST", [128, 104])
        CU8 = sb("CU8", [128, 64], U8)
        SCHW = sb("SCHW", [128, 4096])
        FTJ = sb("FTJ", [128, 4096])
        PWIN = sb("PWIN", [128, 4096])
        V = sb("V", [128, 128])
        VT = sb("VT", [128, 128])
        PH = sb("PH", [128, 4 * S])
        SCE = sb("SCE", [128, 4096])
        CANDB = sb("CANDB", [128, 4096])
        BPU8 = sb("BPU8", [128, 4 * S], U8)
        ALF = sb("ALF", [128, 4 * S])
        PSC = sb("PSC", [128, 4 * S])
        LPP = sb("LPP", [128, 32])
        TLP = sb("TLP", [128, 32])
        CANDP = sb("CANDP", [128, 32])
        MX = sb("MX", [128, 1])
        EQP = sb("EQP", [128, 32])
        SCP = sb("SCP", [128, 32])
        PW = sb("PW", [128, 1])
        P32 = sb("P32", [128, 32])
        T32 = sb("T32", [128, 32])
        PR = sb("PR", [128, 32])
        TEND = sb("TEND", [128, 32])
        TMPD = sb("TMPD", [128, 4])
        MKU8 = sb("MKU8", [128, 4 * S], U8)
        ALFU8 = sb("ALFU8", [128, 4 * S], U8)
        INVU = sb("INVU", [128, 4 * S], U8)
        PU8 = sb("PU8", [128, 4], U8)
        G = sb("G", [128, 4 * S], U8)
        H1 = sb("H1", [128, 2 * S], U8)
        H2 = sb("H2", [128, S], U8)
        H3 = sb("H3", [128, S // 2], U8)
        AFLAT = sb("AFLAT", [128, 16384], U8)
        GT3 = sb("GT3", [128, NM * 32])
        ATAB = sb("ATAB", [128, 4096])
        EQB = sb("EQB", [128, 16384], U8)
        EQC = sb("EQC", [128, 32])
        DEC = sb("DEC", [128, S + 8])
        DECI = sb("DECI", [128, S], I32)

        with (
            nc.semaphore() as dma_sem,
            nc.semaphore() as v_sem,
            nc.Block() as block,
        ):
            @block.sync
            def _(sync):
                for dst, src in ((FT, ft_d), (MKF, mkf_d), (TTI, tti_d),
                                 (TTJ, ttj_d), (CST, cst_d), (CU8, cu8_d)):
                    sync.dma_start(out=dst[:], in_=src[:]).then_inc(dma_sem, 16)
                sync.wait_ge(v_sem, reps)
                sync.dma_start(out=dec_d[:], in_=DECI[:]).then_inc(dma_sem, 16)
                if debug:
                    for dst, src in ((dph_d, PH), (dbp_d, BPU8), (dg_d, G),
                                     (dh1_d, H1), (dh3_d, H3), (ddec_d, DEC),
                                     (dgt3_d, GT3)):
                        sync.dma_start(out=dst[:], in_=src[:]).then_inc(dma_sem, 16)

            def bcast_transpose(v, out_buf, out_off, in_view, K,
                                col_fn=None):
                """out_buf[(q,x), out_off+32k+y] = in_view[(q,y), k-th elem].

                in_view: [128, ...] AP whose free elems number K (strides
                arbitrary). col_fn(k) -> [128, 1] AP for the sim path (defaults
                to 2D column slicing)."""
                if not sim_compat:
                    ish = list(in_view.shape)
                    if len(ish) == 2:
                        ov = out_buf[:, out_off:out_off + 32 * K].rearrange(
                            "p (k y) -> p k y", y=32)
                    elif len(ish) == 3:
                        ov = out_buf[:, out_off:out_off + 32 * K].rearrange(
                            "p (n b y) -> p n b y", b=ish[2], y=32)
                    else:
                        raise AssertionError(ish)
                    v.transpose(out=ov,
                                in_=in_view.unsqueeze(len(ish))
                                    .broadcast_to(ish + [32]))
                else:
                    for k in range(K):
                        col = (col_fn(k) if col_fn is not None
                               else in_view[:, k:k + 1])
                        v.transpose(
                            out=out_buf[:, out_off + 32 * k:out_off + 32 * k + 32],
                            in_=col.broadcast_to([128, 32]))

            @block.vector
            def _(v):
                ctx.enter_context(nc.allow_low_precision(
                    reason="u8 one-hot sums bounded by 31; exact"))
                v.wait_ge(dma_sem, 16 * 6)
                for rep in range(reps):

                    def win_range(w):
                        t0 = 1 + 32 * w
                        return t0, min(t0 + 32, S)

                    def emit_sch_window(w):
                        t0, t1 = win_range(w)
                        K = 4 * (t1 - t0)
                        bcast_transpose(v, FTJ, 0, FT[:, 4 * t0:4 * t0 + K], K)
                        v.drain()
                        v.tensor_tensor(
                            out=SCHW[:, 0:32 * K].rearrange("p (k y) -> p k y",
                                                            y=32),
                            in0=FTJ[:, 0:32 * K].rearrange("p (k y) -> p k y",
                                                           y=32),
                            in1=TTI[:, 0:32].unsqueeze(1).broadcast_to(
                                [128, K, 32]),
                            op=OP.add)
                        v.drain()

                    def emit_bp_window(w):
                        t0, t1 = win_range(w)
                        nt = t1 - t0
                        K = 4 * nt

                        def v4(buf):
                            return buf[:, 0:32 * K].rearrange(
                                "p (t b i) -> p t b i", b=4, i=32)

                        def ph4(c0):
                            return (FT if False else PH)[
                                :, c0:c0 + K].rearrange(
                                "p (t b) -> p t b", b=4).unsqueeze(3) \
                                .broadcast_to([128, nt, 4, 32])

                        bcast_transpose(v, PWIN, 0,
                                        PH[:, 4 * (t0 - 1):4 * (t0 - 1) + K], K)
                        v.tensor_tensor(
                            out=v4(SCE),
                            in0=FT[:, 4 * t0:4 * t0 + K].rearrange(
                                "p (t b) -> p t b", b=4).unsqueeze(3)
                                .broadcast_to([128, nt, 4, 32]),
                            in1=TTJ[:, 0:32].unsqueeze(1).unsqueeze(1)
                                .broadcast_to([128, nt, 4, 32]),
                            op=OP.add)
                        v.drain()
                        v.tensor_tensor(out=v4(CANDB), in0=v4(SCE),
                                        in1=v4(PWIN), op=OP.add)
                        v.drain()
                        v.tensor_tensor(out=v4(SCE), in0=v4(CANDB),
                                        in1=ph4(4 * t0), op=OP.is_equal)
                        v.drain()
                        v.scalar_tensor_tensor(
                            out=CANDB[:, 0:32 * K].rearrange(
                                "p (tb i) -> p tb i", i=32),
                            in0=SCE[:, 0:32 * K].rearrange(
                                "p (tb i) -> p tb i", i=32),
                            scalar=-BIG,
                            in1=CST[:, 32:64].unsqueeze(1).broadcast_to(
                                [128, K, 32]),
                            op0=OP.mult, op1=OP.add)
                        v.drain()
                        v.tensor_reduce(
                            out=BPU8[:, 4 * t0:4 * t0 + K].rearrange(
                                "p (t b) -> p t b", b=4),
                            in_=v4(CANDB), axis=AX.X, op=OP.min)
                        v.drain()

                    # ---- init ----
                    v.memset(BPU8[:], 0)
                    v.memset(P32[:], 0.0)
                    v.memset(LPP[:], 0.0)
                    v.memset(DEC[:], 0.0)
                    v.stream_shuffle(out=TEND[:], in_=TTJ[:], mask=[END] * 32)
                    v.tensor_scalar_add(out=PH[:, 0:4], in0=FT[:, 0:4],
                                        scalar1=TTJ[:, START:START + 1])
                    v.tensor_sub(out=ALF[:], in0=MKF[:, 0:4 * S],
                                 in1=MKF[:, 4:4 * S + 4])
                    v.drain()
                    emit_sch_window(0)

                    # ---- scan ----
                    for t in range(1, S):
                        w, k = (t - 1) // 32, (t - 1) % 32
                        if k == 0 and t > 1:
                            emit_sch_window(w)
                            if phase != 'scan_nobp':
                                emit_bp_window(w - 1)
                        vv = V[:].rearrange("p (b j) -> p b j", b=4)
                        v.tensor_tensor(
                            out=vv,
                            in0=SCHW[:, 128 * k:128 * k + 128].rearrange(
                                "p (b j) -> p b j", b=4),
                            in1=PH[:, 4 * (t - 1):4 * t].unsqueeze(2)
                                .broadcast_to([128, 4, 32]),
                            op=OP.add)
                        if not sim_compat:
                            v.tensor_reduce(out=PH[:, 4 * t:4 * t + 4], in_=vv,
                                            axis=AX.X, op=OP.max,
                                            apply_transpose=True)
                        else:
                            v.drain()
                            v.transpose(out=VT[:, 0:128], in_=V[:, 0:128])
                            v.drain()
                            v.tensor_reduce(out=PH[:, 4 * t:4 * t + 4],
                                            in_=VT[:].rearrange(
                                                "p (b j) -> p b j", b=4),
                                            axis=AX.X, op=OP.max)
                        if not no_drain:
                            v.drain()
                    if phase != 'scan_nobp':
                        emit_bp_window(NW - 1)
                    if phase in ('scan', 'scan_nobp'):
                        v.tensor_copy(out=DECI[:, 0:4], in_=PH[:, 0:4])
                        v.drain().then_inc(v_sem, 1)
                        continue

                    # ---- pointer phase ----
                    v.tensor_scalar(out=PSC[:], in0=ALF[:], scalar1=1.0,
                                    scalar2=1e30, op0=OP.subtract, op1=OP.mult)
                    v.drain()
                    v.tensor_tensor(out=CANDB[:, 0:4 * S], in0=PH[:],
                                    in1=PSC[:], op=OP.add)
                    v.drain()
                    v.tensor_reduce(out=LPP[:, 0:4],
                                    in_=CANDB[:, 0:4 * S].rearrange(
                                        "p (t b) -> p b t", b=4),
                                    axis=AX.X, op=OP.max)
                    v.drain()
                    v.transpose(out=TLP[:], in_=LPP[:])
                    v.drain()
                    v.tensor_tensor(out=CANDP[:], in0=TLP[:], in1=TEND[:],
                                    op=OP.add)
                    v.drain()
                    v.tensor_reduce(out=MX[:], in_=CANDP[:], axis=AX.X,
                                    op=OP.max)
                    v.drain()
                    v.tensor_tensor(out=EQP[:], in0=CANDP[:],
                                    in1=MX[:].broadcast_to([128, 32]),
                                    op=OP.is_equal)
                    v.drain()
                    v.scalar_tensor_tensor(out=SCP[:], in0=EQP[:], scalar=-BIG,
                                           in1=CST[:, 32:64], op0=OP.mult,
                                           op1=OP.add)
                    v.drain()
                    v.tensor_reduce(out=PW[:], in_=SCP[:], axis=AX.X, op=OP.min)
                    v.drain()
                    v.tensor_copy(out=P32[:, 0:1], in_=PW[:])
                    v.drain()
                    v.transpose(out=T32[:], in_=P32[:])
                    v.drain()
                    v.stream_shuffle(out=PR[:], in_=T32[:], mask=[0] * 32)
                    v.drain()
                    v.tensor_tensor(out=TMPD[:], in0=PR[:, 0:4],
                                    in1=CST[:, 96:100], op=OP.mult)
                    v.drain()
                    v.tensor_reduce(out=DEC[:, S:S + 1], in_=TMPD[:], axis=AX.X,
                                    op=OP.add)
                    v.tensor_copy(out=PU8[:], in_=PR[:, 0:4])
                    v.drain()
                    if phase == 'ptr':
                        v.tensor_copy(out=DECI[:, 0:4], in_=PH[:, 0:4])
                        v.drain().then_inc(v_sem, 1)
                        continue

                    # ---- G build ----
                    v.tensor_copy(out=MKU8[:], in_=MKF[:, 4:4 * S + 4])
                    v.tensor_copy(out=ALFU8[:, 0:4 * S - 4],
                                  in_=ALF[:, 0:4 * S - 4])
                    v.drain()
                    v.tensor_scalar(out=INVU[:, 0:4 * S - 4],
                                    in0=ALFU8[:, 0:4 * S - 4], scalar1=0,
                                    scalar2=None, op0=OP.is_equal)
                    v.tensor_tensor(out=G[:, 0:4 * S - 4], in0=BPU8[:, 4:4 * S],
                                    in1=MKU8[:, 0:4 * S - 4], op=OP.mult)
                    v.drain()
                    v.tensor_tensor(out=G[:, 0:4 * S - 4], in0=G[:, 0:4 * S - 4],
                                    in1=INVU[:, 0:4 * S - 4], op=OP.mult)
                    v.tensor_tensor(
                        out=MKU8[:, 0:4 * S - 4].rearrange("p (k b) -> p k b",
                                                           b=4),
                        in0=ALFU8[:, 0:4 * S - 4].rearrange("p (k b) -> p k b",
                                                            b=4),
                        in1=PU8[:].unsqueeze(1).broadcast_to([128, S - 1, 4]),
                        op=OP.mult)
                    v.drain()
                    v.tensor_tensor(out=G[:, 0:4 * S - 4], in0=G[:, 0:4 * S - 4],
                                    in1=MKU8[:, 0:4 * S - 4], op=OP.add)
                    v.tensor_copy(out=G[:, 4 * S - 4:4 * S],
                                  in_=CU8[:, 32:33].broadcast_to([128, 4]))
                    v.drain()

                    # ---- composes (pure DVE) ----
                    def even_view(slab, n_cnt):
                        # columns (2n, br) of a (k,br) slab -> [128, n, 4]
                        v3 = slab[:, 0:8 * n_cnt].rearrange(
                            "p (n two b) -> p n two b", two=2, b=4)[:, :, 0]
                        return v3, (lambda k: slab[:, 8 * (k // 4) + k % 4:
                                                   8 * (k // 4) + k % 4 + 1])

                    def odd_view(slab, n_cnt):
                        v3 = slab[:, 0:8 * n_cnt].rearrange(
                            "p (n two b) -> p n two b", two=2, b=4)[:, :, 1]
                        return v3, (lambda k: slab[:, 8 * (k // 4) + 4 + k % 4:
                                                   8 * (k // 4) + 4 + k % 4 + 1])

                    def compose(b_slab, n_cnt, out_slab):
                        # AFLAT[(q,x), ((n,b), y)] = b_slab[(q,y), (2n, b)]
                        ev3, ecol = even_view(b_slab, n_cnt)
                        for n0 in range(0, n_cnt, 128):
                            nn = min(128, n_cnt - n0)
                            bcast_transpose(
                                v, AFLAT, 0, ev3[:, n0:n0 + nn], 4 * nn,
                                lambda k: ecol(4 * n0 + k))
                            v.drain()
                            eqv = EQB[:, 0:nn * 128].rearrange(
                                "p (n b y) -> p n b y", b=4, y=32)
                            bside = (b_slab[:, 8 * n0:8 * (n0 + nn)]
                                     .rearrange("p (n e) -> p n e", e=8)
                                     [:, :, 4:8])
                            v.tensor_tensor(
                                out=eqv,
                                in0=CU8[:, 0:32].unsqueeze(1).unsqueeze(1)
                                    .broadcast_to([128, nn, 4, 32]),
                                in1=bside.unsqueeze(3).broadcast_to(
                                    [128, nn, 4, 32]),
                                op=OP.is_equal)
                            v.drain()
                            v.tensor_tensor(
                                out=eqv, in0=eqv,
                                in1=AFLAT[:, 0:128 * nn].rearrange(
                                    "p (n b y) -> p n b y", b=4, y=32),
                                op=OP.mult)
                            v.drain()
                            v.tensor_reduce(
                                out=out_slab[:, 4 * n0:4 * (n0 + nn)].rearrange(
                                    "p (n b) -> p n b", b=4),
                                in_=eqv, axis=AX.X, op=OP.add)
                            v.drain()

                    if phase == 'gbuild':
                        v.tensor_copy(out=DECI[:, 0:4], in_=PH[:, 0:4])
                        v.drain().then_inc(v_sem, 1)
                        continue
                    compose(G, S // 2, H1)
                    compose(H1, S // 4, H2)
                    compose(H2, S // 8, H3)

                    # ---- chase/apply tables (example-select from flat) ----
                    def build_table_tile(src_spec, m0, mm, out_tab,
                                         out_off):
                        # AFLAT[(q,x), ((m,b), y)] = src[(q,y), (m0+m, b)]
                        # out_tab[p, out_off+32m+y] =
                        #     sum_b AFLAT[p, (m,b,y)] * SEL[p,b]
                        if isinstance(src_spec, tuple):
                            sv, scol = src_spec
                        else:
                            sv, scol = src_spec, None
                        if len(sv.shape) == 2:
                            svt = sv[:, 4 * m0:4 * (m0 + mm)]
                            scolt = None
                        else:
                            svt = sv[:, m0:m0 + mm]
                            scolt = (lambda k: scol(4 * m0 + k))
                        bcast_transpose(v, AFLAT, 0, svt, 4 * mm, scolt)
                        v.drain()
                        av = AFLAT[:, 0:128 * mm].rearrange(
                            "p (m b y) -> p m y b", b=4, y=32)
                        ev = EQB[:, 0:mm * 128].rearrange(
                            "p (m y b) -> p m y b", y=32, b=4)
                        v.tensor_tensor(
                            out=ev, in0=av,
                            in1=CU8[:, 33:37].unsqueeze(1).unsqueeze(1)
                                .broadcast_to([128, mm, 32, 4]),
                            op=OP.mult)
                        v.drain()
                        v.tensor_reduce(
                            out=out_tab[:, out_off:out_off + 32 * mm]
                                .rearrange("p (m y) -> p m y", y=32),
                            in_=ev, axis=AX.X, op=OP.add)
                        v.drain()

                    for m0 in range(0, S // 8, 128):
                        mm = min(128, S // 8 - m0)
                        build_table_tile(H3[:, 0:4 * (S // 8)], m0, mm,
                                         GT3, 32 * m0)

                    # ---- chase ----
                    for m in range(NM - 1, -1, -1):
                        v.scalar_tensor_tensor(
                            out=EQC[:], in0=CST[:, 0:32],
                            scalar=DEC[:, 8 * m + 8:8 * m + 9],
                            in1=GT3[:, 32 * m:32 * m + 32],
                            op0=OP.is_equal, op1=OP.mult,
                            accum_out=DEC[:, 8 * m:8 * m + 1])
                        v.drain()

                    # ---- back-substitution ----
                    def apply_level(cnt, stride, out_start, src_start,
                                    src_spec):
                        for m0 in range(0, cnt, 128):
                            mm = min(128, cnt - m0)
                            build_table_tile(src_spec, m0, mm, ATAB, 0)
                            eqv = SCE[:, 0:mm * 32].rearrange(
                                "p (m y) -> p m y", y=32)
                            srcv = (DEC[:, src_start + stride * m0:
                                        src_start + stride * (m0 + mm)]
                                    .rearrange("p (m e) -> p m e", e=stride)
                                    [:, :, 0:1].broadcast_to([128, mm, 32]))
                            v.tensor_tensor(
                                out=eqv,
                                in0=CST[:, 0:32].unsqueeze(1).broadcast_to(
                                    [128, mm, 32]),
                                in1=srcv, op=OP.is_equal)
                            v.drain()
                            v.tensor_tensor(
                                out=eqv, in0=eqv,
                                in1=ATAB[:, 0:32 * mm].rearrange(
                                    "p (m y) -> p m y", y=32),
                                op=OP.mult)
                            v.drain()
                            v.tensor_reduce(
                                out=(DEC[:, out_start + stride * m0:
                                         out_start + stride * (m0 + mm)]
                                     .rearrange("p (m e) -> p m e", e=stride)
                                     [:, :, 0:1]),
                                in_=eqv, axis=AX.X, op=OP.add)
                            v.drain()

                    apply_level(S // 8, 8, 4, 8, odd_view(H2, S // 8))
                    apply_level(S // 4, 4, 2, 4, odd_view(H1, S // 4))
                    apply_level(S // 2, 2, 1, 2, odd_view(G, S // 2))

                    v.tensor_copy(out=DECI[:], in_=DEC[:, 0:S])
                    v.drain().then_inc(v_sem, 1)

    return nc


def pack_inputs(feats, transitions, mask, S):
    """Host-side layout packing (pure data movement + dtype conversion)."""
    trans = np.ascontiguousarray(np.asarray(transitions, np.float32))
    tti = np.ascontiguousarray(np.tile(trans, (4, 1)))       # [(q,i), j]
    ttj = np.ascontiguousarray(np.tile(trans.T, (4, 1)))     # [(q,j), i]
    iota = np.arange(32, dtype=np.float32)
    cst = np.zeros((128, 104), np.float32)
    cst[:, 0:32] = iota[None, :]
    cst[:, 32:64] = iota[None, :] + BIG
    p = np.arange(128)
    for br in range(4):
        cst[:, 96 + br] = (p % 4 == br).astype(np.float32)
    cu8 = np.zeros((128, 64), np.uint8)
    cu8[:, 0:32] = np.arange(32, dtype=np.uint8)[None, :]
    cu8[:, 32] = (p % 32).astype(np.uint8)
    for br in range(4):
        cu8[:, 33 + br] = (p % 4 == br).astype(np.uint8)
    in_maps = []
    bc = 16
    for c in range(NCORES):
        f = np.asarray(feats[bc * c:bc * c + bc], np.float32)  # [16, S, 32]
        ft = np.ascontiguousarray(
            f.reshape(4, 4, S, T).transpose(0, 3, 2, 1).reshape(128, 4 * S))
        m = np.asarray(mask[bc * c:bc * c + bc]).astype(np.float32)  # [16, S]
        mk = np.broadcast_to(
            m.reshape(4, 1, 4, S).transpose(0, 1, 3, 2), (4, 32, S, 4))
        mkp = np.zeros((128, 4 * S + 4), np.float32)
        mkp[:, :4 * S] = mk.reshape(128, 4 * S)
        in_maps.append({"ft": ft, "mkf": mkp, "tti": tti, "ttj": ttj,
                        "cst": cst, "cu8": cu8})
    return in_maps


def unpack_outputs(results, S):
    out = np.empty((128, S), np.int32)
    bc = 16
    for c in range(NCORES):
        d = np.asarray(results[c]["dec"]).reshape(4, 32, S)
        out[bc * c:bc * c + bc] = d[:, 0:4, :].reshape(16, S)
    return out


_NC_CACHE = {}


def kernel(feats, transitions, mask):
    B, S, Tin = feats.shape
    assert (B, Tin) == (128, 32)
    if S not in _NC_CACHE:
        _NC_CACHE[S] = build_nc2(S)
    nc = _NC_CACHE[S]
    in_maps = pack_inputs(feats, transitions, mask, S)
    res = run_bass_kernel_spmd(nc, in_maps, list(range(NCORES)))
    return unpack_outputs(res.results, S)
